# revision 1
# baseline (speedup 1.0000x reference)
"""TRN2 Bass kernel for the NTK-track Conv1d problem.

Reference computation (per batch element b, all fp32):
    xv = relu(x[...,0]); x0 = relu(x[...,1]); dx = x[...,2] * (x[...,1] >= 0)
    s = sqrt(|alpha|)  (per-tap scale, K=9)
    x_out  = conv1d(xv, weight*s)/sqrt(C) + bias*sqrt(|beta|)
    x0_out = conv1d(x0, w0*s)/sqrt(C)     + b0*sqrt(|beta|)
    dx_out = (conv1d(dx, w0*s) + conv1d(x0, w*s))/sqrt(C) + b*sqrt(|beta|)
    out = stack([x_out, x0_out, dx_out], -1)

Shapes: x (8, 256, 8192, 3); weight/w0/w (256, 256, 9); pad=4 (same conv).

Strategy: data-parallel over batch (8 cores, 1 batch element each).
Per core, conv1d(track, W) is computed as 9*2 shifted 128x128x512 matmuls
accumulated in PSUM (contraction over C and tap k); operands are float32r
(TF32-like fast PE mode, full speed at free-dim 512, ~1e-3 rel err).
The 1/sqrt(C) and sqrt(|alpha|) factors are folded into the weights on the
host; the sqrt(|beta|)-scaled biases are added during PSUM->SBUF eviction.
"""

import math

import numpy as np

B, C, O, T, K = 8, 256, 256, 8192, 9
PAD = 4
P = 128  # partitions
TT = 512  # time-tile (matmul free dim)
NT = T // TT  # 16 time tiles
CCH = C // P  # 2 contraction chunks
OCH = O // P  # 2 output-partition chunks
HALO = TT + 2 * PAD  # 520 input columns per tile
NCORES = 8


def _split_excess_waits(nc) -> int:
    """Move excess per-instruction semaphore waits onto standalone
    EventSemaphore carrier instructions.

    The walrus build in this environment rejects any instruction carrying
    more than ONE sync wait at codegen ("Too many sync wait commands");
    Tile's sem assignment freely emits several. Walk the finished BIR and
    hoist overflow waits onto fresh same-engine EventSemaphore instructions
    placed immediately before the over-budget instruction.
    """
    import concourse.mybir as mybir

    n_carriers = 0
    for f in nc.m.functions:
        for blk in f.blocks:
            insts = list(blk.instructions)
            new_insts = []
            dirty = False
            for inst in insts:
                si = inst.sync_info
                waits = list(si.on_wait) if si is not None and si.on_wait else []
                if len(waits) > 1:
                    overflow, keep = waits[:-1], waits[-1:]
                    for w in overflow:
                        ev = mybir.InstEventSemaphore(
                            name=f"{inst.name}_waitc{n_carriers}",
                            engine=inst.engine,
                        )
                        ev.sync_info = mybir.SyncInfo(on_wait=[w], on_update=[])
                        nc.register_instruction(ev, overwrite=True)
                        new_insts.append(ev)
                        n_carriers += 1
                    upd = list(si.on_update) if si.on_update else []
                    inst.sync_info = mybir.SyncInfo(on_wait=keep, on_update=upd)
                    dirty = True
                new_insts.append(inst)
            if dirty:
                blk.instructions = new_insts
    return n_carriers


def _dedupe_ldweights(nc) -> int:
    """Drop an InstLdweights whose weights AP matches the previous kept
    InstLdweights on the same stream with only Matmult / EventSemaphore
    instructions in between (the PE array still holds those weights).
    Waits from a dropped LDW migrate to the next kept PE instruction.
    Must run BEFORE _split_excess_waits so merged waits get re-split."""
    import concourse.mybir as mybir

    removed = 0
    for f in nc.m.functions:
        for blk in f.blocks:
            insts = list(blk.instructions)
            new_insts = []
            last_ld_key = None
            pend_waits = []
            for inst in insts:
                op = inst.opcode
                if op == "Ldweights":
                    key = str(inst.ins[0])
                    if key == last_ld_key:
                        si = inst.sync_info
                        if si is not None and si.on_wait:
                            pend_waits.extend(list(si.on_wait))
                        if si is not None and si.on_update:
                            # don't drop an LDW other procs wait on
                            new_insts.append(inst)
                            continue
                        removed += 1
                        continue
                    last_ld_key = key
                elif op in ("Matmult", "EventSemaphore"):
                    pass  # doesn't clobber the loaded weights
                else:
                    last_ld_key = None
                if pend_waits and inst.engine == mybir.EngineType.PE:
                    si = inst.sync_info
                    w = list(si.on_wait) if si is not None and si.on_wait else []
                    u = list(si.on_update) if si is not None and si.on_update else []
                    inst.sync_info = mybir.SyncInfo(on_wait=pend_waits + w, on_update=u)
                    pend_waits = []
                new_insts.append(inst)
            if removed:
                assert not pend_waits
                blk.instructions = new_insts
    return removed


def _build_nc(reps: int = 1, w_dt: str = "float32r", r_dt: str = "float32r",
              skip_mm: bool = False, skip_post: bool = False, skip_pre: bool = False,
              pair_t: bool = False):
    import concourse.bass as bass
    import concourse.mybir as mybir
    from concourse.tile import TileContext

    f32 = mybir.dt.float32
    wdt = getattr(mybir.dt, w_dt)   # weights (stationary operand) dtype
    rdt = getattr(mybir.dt, r_dt)   # tracks (moving operand) dtype
    AF = mybir.ActivationFunctionType
    OP = mybir.AluOpType

    nc = bass.Bass()
    xd = nc.declare_dram_parameter("xd", [C, T * 3], f32, isOutput=False)
    w1 = nc.declare_dram_parameter("w1", [P, CCH * K * OCH * P], wdt, isOutput=False)
    w2 = nc.declare_dram_parameter("w2", [P, CCH * K * OCH * P], wdt, isOutput=False)
    w3 = nc.declare_dram_parameter("w3", [P, CCH * K * OCH * P], wdt, isOutput=False)
    bs = nc.declare_dram_parameter("bs", [P, OCH * 3], f32, isOutput=False)
    yd = nc.declare_dram_parameter("yd", [C, T * 3], f32, isOutput=True)

    with TileContext(nc) as tc:
        with (
            tc.tile_pool(name="wpool", bufs=1) as wpool,
            tc.tile_pool(name="slabs", bufs=4 if not pair_t else 6) as slabs,
            tc.tile_pool(name="trks", bufs=4 if not pair_t else 6) as trks,
            tc.tile_pool(name="opool", bufs=4) as opool,
            tc.tile_pool(name="psum", bufs=2, space="PSUM") as psp,
            tc.tile_pool(name="psumx", bufs=2, space="PSUM") as pspx,
            tc.tile_pool(name="psum1", bufs=1, space="PSUM") as psp1,
        ):
            # Persistent weights / biases
            w1s = wpool.tile([P, CCH, K, OCH, P], wdt)
            w2s = wpool.tile([P, CCH, K, OCH, P], wdt)
            w3s = wpool.tile([P, CCH, K, OCH, P], wdt)
            bss = wpool.tile([P, OCH, 3], f32)
            nc.sync.dma_start(w1s[:], w1[:].rearrange("p (c k o q) -> p c k o q", c=CCH, k=K, o=OCH))
            nc.sync.dma_start(w2s[:], w2[:].rearrange("p (c k o q) -> p c k o q", c=CCH, k=K, o=OCH))
            nc.sync.dma_start(w3s[:], w3[:].rearrange("p (c k o q) -> p c k o q", c=CCH, k=K, o=OCH))
            nc.sync.dma_start(bss[:], bs[:].rearrange("p (o s) -> p o s", o=OCH))

            def make_tracks(tt):
                t0 = tt * TT
                tracks = []
                for cc in range(CCH):
                    slab = slabs.tile([P, HALO * 3], f32, tag="slab")
                    lo = 3 * (t0 - PAD)
                    hi = 3 * (t0 + TT + PAD)
                    zlo = max(0, -lo)      # zero-pad columns at the left edge
                    zhi = max(0, hi - 3 * T)  # and at the right edge
                    if zlo:
                        nc.vector.memset(slab[:, :zlo], 0.0)
                    if zhi:
                        nc.vector.memset(slab[:, HALO * 3 - zhi:], 0.0)
                    nc.sync.dma_start(
                        slab[:, zlo : HALO * 3 - zhi],
                        xd[cc * P : (cc + 1) * P, lo + zlo : hi - zhi],
                    )
                    sv = slab[:].rearrange("p (t s) -> p t s", s=3)
                    trk = trks.tile([P, 3, HALO], rdt, tag="trk")
                    if skip_pre:
                        nc.vector.tensor_copy(trk[:, 0], sv[:, :HALO, 0])
                    else:
                        # xv = relu(track0); x0 = relu(track1)  (ACT engine)
                        nc.scalar.activation(trk[:, 0], sv[:, :, 0], AF.Relu)
                        nc.scalar.activation(trk[:, 1], sv[:, :, 1], AF.Relu)
                        # dx = track2 * (track1 >= 0)  (DVE engine)
                        msk = trks.tile([P, HALO], f32, tag="msk")
                        nc.vector.tensor_scalar(msk[:], sv[:, :, 1], 0.0, None, OP.is_ge)
                        nc.vector.tensor_tensor(trk[:, 2], msk[:], sv[:, :, 2], OP.mult)
                    tracks.append(trk)
                return tracks

            def post(oc, t0, ps_x, ps_x0, ps_dx):
                ot = opool.tile([P, TT, 3], f32, tag="ot")
                nc.vector.tensor_scalar_add(ot[:, :, 0], ps_x[:], bss[:, oc, 0:1])
                nc.vector.tensor_scalar_add(ot[:, :, 1], ps_x0[:], bss[:, oc, 1:2])
                nc.vector.tensor_scalar_add(ot[:, :, 2], ps_dx[:], bss[:, oc, 2:3])
                nc.sync.dma_start(
                    yd[oc * P : (oc + 1) * P, 3 * t0 : 3 * (t0 + TT)],
                    ot[:].rearrange("p t s -> p (t s)"),
                )

            def body_pair(_iv=None):
                # two time-tiles per weight pass: 8 matmuls per 3 weight loads
                for tp in range(NT // 2):
                    tts = (2 * tp, 2 * tp + 1)
                    tr2 = [make_tracks(tt) for tt in tts]
                    for oc in range(OCH):
                        psx = [pspx.tile([P, TT], f32, tag=f"psx{j}", name=f"psx{j}") for j in range(2)]
                        ps0 = [psp1.tile([P, TT], f32, tag=f"ps0{j}", name=f"ps0{j}") for j in range(2)]
                        psd = [psp1.tile([P, TT], f32, tag=f"psd{j}", name=f"psd{j}") for j in range(2)]
                        for cc in range(CCH):
                            for k in range(K):
                                first = cc == 0 and k == 0
                                last = cc == CCH - 1 and k == K - 1
                                for j in (0, 1):
                                    nc.tensor.matmul(
                                        psx[j][:], w1s[:, cc, k, oc],
                                        tr2[j][cc][:, 0, k : k + TT],
                                        start=first, stop=last)
                                for j in (0, 1):
                                    nc.tensor.matmul(
                                        ps0[j][:], w2s[:, cc, k, oc],
                                        tr2[j][cc][:, 1, k : k + TT],
                                        start=first, stop=last)
                                for j in (0, 1):
                                    nc.tensor.matmul(
                                        psd[j][:], w2s[:, cc, k, oc],
                                        tr2[j][cc][:, 2, k : k + TT],
                                        start=first, stop=False)
                                for j in (0, 1):
                                    nc.tensor.matmul(
                                        psd[j][:], w3s[:, cc, k, oc],
                                        tr2[j][cc][:, 1, k : k + TT],
                                        start=False, stop=last)
                        if skip_post:
                            continue
                        for j in (0, 1):
                            post(oc, tts[j] * TT, psx[j], ps0[j], psd[j])

            def body(_iv=None):
                for tt in range(NT):
                    t0 = tt * TT
                    tracks = make_tracks(tt)
                    for oc in range(OCH):
                        ps_x = psp.tile([P, TT], f32, tag="psx")
                        ps_x0 = psp.tile([P, TT], f32, tag="psx0")
                        ps_dx = psp.tile([P, TT], f32, tag="psdx")
                        if skip_mm:
                            nc.tensor.matmul(ps_x[:], w1s[:, 0, 0, oc], tracks[0][:, 0, 0:TT], start=True, stop=True)
                            nc.tensor.matmul(ps_x0[:], w2s[:, 0, 0, oc], tracks[0][:, 1, 0:TT], start=True, stop=True)
                            nc.tensor.matmul(ps_dx[:], w2s[:, 0, 0, oc], tracks[0][:, 2, 0:TT], start=True, stop=True)
                        else:
                            # serial groups: each conv's matmuls wait only on
                            # the weight tensor(s) it needs
                            for cc in range(CCH):
                                for k in range(K):
                                    nc.tensor.matmul(
                                        ps_x[:], w1s[:, cc, k, oc],
                                        tracks[cc][:, 0, k : k + TT],
                                        start=(cc == 0 and k == 0),
                                        stop=(cc == CCH - 1 and k == K - 1),
                                    )
                            for cc in range(CCH):
                                for k in range(K):
                                    nc.tensor.matmul(
                                        ps_x0[:], w2s[:, cc, k, oc],
                                        tracks[cc][:, 1, k : k + TT],
                                        start=(cc == 0 and k == 0),
                                        stop=(cc == CCH - 1 and k == K - 1),
                                    )
                            for cc in range(CCH):
                                for k in range(K):
                                    nc.tensor.matmul(
                                        ps_dx[:], w2s[:, cc, k, oc],
                                        tracks[cc][:, 2, k : k + TT],
                                        start=(cc == 0 and k == 0),
                                        stop=False,
                                    )
                                    nc.tensor.matmul(
                                        ps_dx[:], w3s[:, cc, k, oc],
                                        tracks[cc][:, 1, k : k + TT],
                                        start=False,
                                        stop=(cc == CCH - 1 and k == K - 1),
                                    )
                        if skip_post:
                            continue
                        post(oc, t0, ps_x, ps_x0, ps_dx)

            main = body_pair if pair_t else body
            if reps == 1:
                main()
            else:
                with tc.For_i(0, reps, 1) as _i:
                    main(_i)

    ndedup = _dedupe_ldweights(nc)
    if ndedup:
        import logging
        logging.getLogger(__name__).info("deduped %d ldweights", ndedup)
    _split_excess_waits(nc)
    return nc


_CACHE: dict = {}


def _prep_weights(weight, w0, w, alpha):
    """(O, C, K) fp32 -> lhsT layout [c_lo, c_chunk, k, o_chunk, o_lo] flat."""
    s = np.sqrt(np.abs(np.asarray(alpha, np.float32)))  # (1,1,K)
    inv_sqrt_c = np.float32(1.0 / math.sqrt(C))
    out = []
    for wt in (weight, w0, w):
        wt = np.asarray(wt, np.float32) * s * inv_sqrt_c  # (O, C, K)
        wt = wt.reshape(OCH, P, CCH, P, K).transpose(3, 2, 4, 0, 1)
        out.append(np.ascontiguousarray(wt).reshape(P, CCH * K * OCH * P))
    return out


def kernel(x, weight, w0, w, alpha, bias, b0, b, beta):
    from concourse.bass_utils import run_bass_kernel_spmd

    x = np.asarray(x, np.float32)
    w1_np, w2_np, w3_np = _prep_weights(weight, w0, w, alpha)
    sb = np.float32(math.sqrt(abs(float(np.asarray(beta)))))
    biases = np.stack(
        [np.asarray(bias, np.float32) * sb,
         np.asarray(b0, np.float32) * sb,
         np.asarray(b, np.float32) * sb],
        axis=-1,
    )  # (O, 3) in track order [x, x0, dx]
    bs_np = np.ascontiguousarray(biases.reshape(OCH, P, 3).transpose(1, 0, 2)).reshape(
        P, OCH * 3
    )

    if "nc" not in _CACHE:
        _CACHE["nc"] = _build_nc()
    nc = _CACHE["nc"]

    in_maps = []
    for c in range(NCORES):
        in_maps.append(
            {
                "xd": np.ascontiguousarray(x[c].reshape(C, T * 3)),
                "w1": w1_np,
                "w2": w2_np,
                "w3": w3_np,
                "bs": bs_np,
            }
        )
    res = run_bass_kernel_spmd(nc, in_maps, list(range(NCORES)))
    out = np.empty((B, C, T, 3), np.float32)
    for c in range(NCORES):
        out[c] = res.results[c]["yd"].reshape(C, T, 3)
    return out

